# revision 2
# baseline (speedup 1.0000x reference)
"""GQA causal attention (B=2, T=2048, C=2048, 32 Q heads, 8 KV heads) on 8
Trainium2 NeuronCores — v2.

Sharding: tensor-parallel over KV-head groups (core i owns KV head i and its
4 query heads). All matmuls in bf16 (f32 PSUM accumulation): measured end-to-
end rel-err ~4e-3 vs the 2e-2 gate.

v2 structure (vs the f32r v1):
  - bf16 everywhere halves DMA and enables 1024-col moving operands + FWL.
  - Attention packs all 4 query heads into one scores tile [k, (pair,head,q)]
    so each kv-block stationary serves 1024-col matmuls; exp runs on 2048-col
    ACTIVATEs (the Act engine is the phase bottleneck at ~(N+352)/1.2 ns).
  - Causal diagonal blocks are trimmed at 128-col granularity (MMs, exp and
    the AV accumulation all skip the dead quarter-triangle).
  - Normalize uses a selector-matmul broadcast of 1/l (no per-row DMAs).
  - The AllGather is split per batch and overlaps the other batch's compute;
    the output projection runs transposed (out [oc, tok]) so wp chunks are
    stationary and yt tiles stream as 1024-col moving operands. Host
    transposes the final result.
"""

import sys

sys.path.insert(0, "/opt/trn_rl_repo")

import numpy as np
import ml_dtypes

import concourse.bass as bass
import concourse.mybir as mybir
import concourse.tile as tile

P = 128
B, T, C = 2, 2048, 2048
BT = B * T            # 4096
NH, NKV = 32, 8
HD = C // NH          # 64
G = NH // NKV         # 4 q heads per core
CQ = G * HD           # 256 q/out channels per core
KC = C // P           # 16 contraction chunks
TB = 1024             # phase-1/4 token block
TQ = 512              # attention q-chunk
NCORES = 8
NKB = T // P          # 16 kv blocks per batch

f32 = mybir.dt.float32
bf16 = mybir.dt.bfloat16
EXP = mybir.ActivationFunctionType.Exp
SCALE = float(HD) ** -0.5


def split_multi_waits(nc):
    """Walrus codegen allows only one sync-wait per engine instruction; move
    extras onto standalone same-engine EventSemaphore waits placed before."""
    for fn in nc.m.functions:
        for bb in fn.blocks:
            out = []
            for inst in bb.instructions:
                si = inst.sync_info
                if si is not None and si.on_wait and len(si.on_wait) > 1:
                    waits = list(si.on_wait)
                    for j, w in enumerate(waits[:-1]):
                        nop = mybir.InstEventSemaphore(
                            name=f"{inst.name}-ws{j}", ins=[], outs=[],
                            engine=inst.engine)
                        nop.sync_info = mybir.SyncInfo(on_wait=[w], on_update=[])
                        out.append(nop)
                    inst.sync_info = mybir.SyncInfo(
                        on_wait=[waits[-1]], on_update=list(si.on_update))
                out.append(inst)
            try:
                bb.instructions[:] = out
            except TypeError:
                bb.instructions.clear()
                bb.instructions.extend(out)


def build(phases=(1, 2, 3, 4)):
    nc = bass.Bass(num_devices=NCORES)

    xt_d = nc.dram_tensor("xt", [C, BT], bf16, kind="ExternalInput")
    wq_d = nc.dram_tensor("wq", [C, CQ], bf16, kind="ExternalInput")
    wkv_d = nc.dram_tensor("wkv", [C, P], bf16, kind="ExternalInput")
    wp_d = nc.dram_tensor("wp", [C, CQ], bf16, kind="ExternalInput")
    bpt_d = nc.dram_tensor("bpt", [1, CQ], bf16, kind="ExternalInput")
    ones_d = nc.dram_tensor("ones", [1, TB], bf16, kind="ExternalInput")
    mask_d = nc.dram_tensor("mask4", [P, 4 * P], bf16, kind="ExternalInput")
    esel_d = nc.dram_tensor("esel", [32, 32 * HD], bf16, kind="ExternalInput")
    idn_d = nc.dram_tensor("ident", [P, P], bf16, kind="ExternalInput")
    out_d = nc.dram_tensor("out", [CQ, BT], f32, kind="ExternalOutput")

    with tile.TileContext(nc) as tc, \
         nc.allow_low_precision(reason="bf16 pipeline validated vs 2e-2 gate"):
        with tc.tile_pool(name="res", bufs=1) as res, \
             tc.tile_pool(name="dram", bufs=1, space="DRAM") as dp:
            # constants / weights
            wq_sb = res.tile([P, KC, CQ], bf16)
            nc.sync.dma_start(wq_sb[:], wq_d.rearrange("(o p) n -> p o n", p=P))
            wkv_sb = res.tile([P, KC, P], bf16)
            nc.scalar.dma_start(wkv_sb[:], wkv_d.rearrange("(o p) n -> p o n", p=P))
            idn_sb = res.tile([P, P], bf16)
            nc.scalar.dma_start(idn_sb[:], idn_d[:, :])
            mask_sb = res.tile([P, 4 * P], bf16)
            nc.scalar.dma_start(mask_sb[:], mask_d[:, :])
            esel_sb = res.tile([32, 32 * HD], bf16)
            nc.scalar.dma_start(esel_sb[:], esel_d[:, :])
            ones_sb = res.tile([1, TB], bf16)
            nc.scalar.dma_start(ones_sb[:], ones_d[:, :])
            bpt_sb = res.tile([1, CQ], bf16)
            nc.scalar.dma_start(bpt_sb[:], bpt_d[:, :])
            wp_sb = res.tile([P, KC, CQ], bf16)

            # long-lived activations (both batches)
            # qP[pr]: [64, b, h, qc, TQ] for the head pair pr (heads 2pr, 2pr+1)
            qP = [res.tile([HD, B, 2, 4, TQ], bf16, name=f"qp{pr}")
                  for pr in range(2)]
            kT = res.tile([HD, B, NKB, P], bf16)
            va = res.tile([P, B, NKB, HD + 2], bf16)   # v natural + ones col
            yU = res.tile([HD + 1, B, 4, 4 * TQ], bf16)  # unnormalized y + l row
            lA = res.tile([32, TQ], bf16)              # l rows, slot-major
            rA = res.tile([32, TQ], bf16)
            yt_loc = [dp.tile([CQ, T], bf16, name=f"ytl{b}") for b in range(B)]
            yt_ag = [dp.tile([NCORES * CQ, T], bf16, addr_space="Shared",
                             name=f"ytag{b}") for b in range(B)]

            yn_all = res.tile([HD, 4, 4, TQ], bf16)
            nc.vector.memset(va[:, :, :, HD:HD + 1], 1.0)
            nc.vector.memset(lA[:, :], 1.0)

            # PE warmup: dummy matmuls on uninitialized SBUF ramp the HAM
            # clock gate to 8/8 while the first weight/x DMAs are in flight.
            warm_sb = res.tile([P, TQ], bf16)
            nc.vector.memset(warm_sb[:, :], 1.0)
            with tc.tile_pool(name="wps", bufs=1, space="PSUM") as wps:
                w_ps = wps.tile([P, TQ], f32)
                for _ in range(14):
                    nc.tensor.matmul(w_ps[:], warm_sb[:, 0:P], warm_sb[:],
                                     start=True, stop=True,
                                     skip_group_check=True)
            nc.vector.memset(va[:, :, :, HD + 1:HD + 2], 0.0)

            for b in range(B):
                if 1 not in phases:
                    break
                # ---- Phase 1: q/k/v projections for batch b ----
                with tc.tile_pool(name=f"xp{b}", bufs=5) as xp, \
                     tc.tile_pool(name=f"pps{b}", bufs=1, space="PSUM") as pps, \
                     tc.tile_pool(name=f"tps{b}", bufs=2, space="PSUM") as tps:
                    for tb in range(T // TB):
                        q01_ps = pps.tile([P, TB], f32, tag="q01")
                        q23_ps = pps.tile([P, TB], f32, tag="q23")
                        kv_ps = pps.tile([P, TB], f32, tag="kv")
                        for c4 in range(KC // 4):
                            xt_t = xp.tile([P, 4, TB], bf16, tag="xt")
                            eng = nc.sync if c4 % 2 == 0 else nc.scalar
                            eng.dma_start(
                                xt_t[:],
                                xt_d[c4 * 4 * P:(c4 + 1) * 4 * P,
                                     b * T + tb * TB: b * T + (tb + 1) * TB]
                                .rearrange("(o p) n -> p o n", p=P))
                            for ci in range(4):
                                c = c4 * 4 + ci
                                for hf in range(2):
                                    fs = slice(hf * TQ, (hf + 1) * TQ)
                                    nc.tensor.matmul(q01_ps[:, fs],
                                                     wq_sb[:, c, 0:P],
                                                     xt_t[:, ci, fs],
                                                     start=(c == 0),
                                                     stop=(c == KC - 1))
                                    nc.tensor.matmul(q23_ps[:, fs],
                                                     wq_sb[:, c, P:CQ],
                                                     xt_t[:, ci, fs],
                                                     start=(c == 0),
                                                     stop=(c == KC - 1))
                                    nc.tensor.matmul(kv_ps[:, fs],
                                                     wkv_sb[:, c, :],
                                                     xt_t[:, ci, fs],
                                                     start=(c == 0),
                                                     stop=(c == KC - 1))
                        qsl = (slice(None), b, slice(None),
                               slice(2 * tb, 2 * tb + 2), slice(None))
                        nc.vector.tensor_copy(
                            qP[0][:, b, 0, 2 * tb:2 * tb + 2, :].rearrange(
                                "p a n -> p (a n)"), q01_ps[0:HD, :])
                        nc.vector.tensor_copy(
                            qP[0][:, b, 1, 2 * tb:2 * tb + 2, :].rearrange(
                                "p a n -> p (a n)"), q01_ps[HD:P, :])
                        nc.vector.tensor_copy(
                            qP[1][:, b, 0, 2 * tb:2 * tb + 2, :].rearrange(
                                "p a n -> p (a n)"), q23_ps[0:HD, :])
                        nc.vector.tensor_copy(
                            qP[1][:, b, 1, 2 * tb:2 * tb + 2, :].rearrange(
                                "p a n -> p (a n)"), q23_ps[HD:P, :])
                        nc.vector.tensor_copy(
                            kT[:, b, 8 * tb:8 * (tb + 1), :].rearrange(
                                "p k n -> p (k n)"),
                            kv_ps[0:HD, :])
                        vs_t = xp.tile([HD, TB], bf16, tag="vs")
                        nc.vector.tensor_copy(vs_t[:], kv_ps[HD:P, :])
                        for k8 in range(TB // P):
                            kb = tb * (TB // P) + k8
                            vt_ps = tps.tile([P, HD], bf16, tag="vt")
                            nc.tensor.transpose(
                                vt_ps[:], vs_t[:, k8 * P:(k8 + 1) * P],
                                idn_sb[0:HD, 0:HD])
                            nc.vector.tensor_copy(va[:, b, kb, 0:HD], vt_ps[:])

                if 2 not in phases:
                    continue
                # ---- Phase 2: attention for batch b, all 4 heads packed ----
                with tc.tile_pool(name=f"aps{b}", bufs=2, space="PSUM") as aps, \
                     tc.tile_pool(name=f"yps{b}", bufs=1, space="PSUM") as yps, \
                     tc.tile_pool(name=f"ep{b}", bufs=4) as ep:
                    for qc in range(4):
                        y_ps = yps.tile([HD + 2, 4, TQ], f32, tag="y")
                        nkb = 4 * qc + 4
                        for kb in range(nkb):
                            j = kb - 4 * qc
                            cols = slice(0, TQ) if j < 1 else slice(j * P, TQ)
                            sps = []
                            exs = []
                            for pr in range(2):
                                s_ps = aps.tile([P, 2, TQ], f32, tag="s")
                                for h in range(2):
                                    nc.tensor.matmul(
                                        s_ps[:, h, cols],
                                        kT[:, b, kb, :],
                                        qP[pr][:, b, h, qc, cols],
                                        start=True, stop=True,
                                        skip_group_check=True)
                                sps.append(s_ps)
                            for pr in range(2):
                                ex = ep.tile([P, 2, TQ], bf16, tag="ex")
                                nc.scalar.activation(ex[:, :, cols],
                                                     sps[pr][:, :, cols], EXP,
                                                     scale=SCALE)
                                if j >= 0:
                                    nc.vector.tensor_mul(
                                        ex[:, :, j * P:(j + 1) * P],
                                        ex[:, :, j * P:(j + 1) * P],
                                        mask_sb[:, 0:2 * P].rearrange(
                                            "p (g n) -> p g n", g=2))
                                exs.append(ex)
                            for pr in range(2):
                                for h in range(2):
                                    nc.tensor.matmul(
                                        y_ps[:, 2 * pr + h, cols],
                                        va[:, b, kb, :],
                                        exs[pr][:, h, cols],
                                        start=(kb == 0),
                                        stop=(kb == nkb - 1),
                                        skip_group_check=True)
                        nc.vector.tensor_copy(
                            yU[:, b, qc, :].rearrange("p (g n) -> p g n", g=4),
                            y_ps[0:HD + 1, :, :])
                        for hh in range(4):
                            slot = (b * 4 + qc) * 4 + hh
                            nc.sync.dma_start(
                                lA[slot:slot + 1, :],
                                yU[HD:HD + 1, b, qc,
                                   hh * TQ:(hh + 1) * TQ])

                if 3 not in phases:
                    continue
                # ---- Phase 3: normalize + per-batch AllGather ----
                with tc.tile_pool(name=f"np{b}", bufs=4) as npo, \
                     tc.tile_pool(name=f"bps{b}", bufs=2, space="PSUM") as bps:
                    if b == 0:
                        nc.scalar.dma_start(
                            wp_sb[:], wp_d.rearrange("(o p) n -> p o n", p=P))
                    nc.vector.reciprocal(rA[:, :], lA[:, :])
                    for qc in range(4):
                        for hh in range(4):
                            slot = b * 16 + qc * 4 + hh
                            qh = hh  # head index within core (pr*2+h)
                            bc_ps = bps.tile([HD, TQ], f32, tag="bc")
                            nc.tensor.matmul(
                                bc_ps[:], esel_sb[:, slot * HD:(slot + 1) * HD],
                                rA[:, :], start=True, stop=True)
                            nc.vector.tensor_mul(
                                yn_all[:, qh, qc, :],
                                yU[0:HD, b, qc, hh * TQ:(hh + 1) * TQ],
                                bc_ps[:])
                    nc.scalar.dma_start(
                        yt_loc[b][:, :].rearrange(
                            "(qh d) (qc c) -> d qh qc c", d=HD, c=TQ),
                        yn_all[:, :, :, :])
                    nc.gpsimd.collective_compute(
                        "AllGather", mybir.AluOpType.bypass,
                        replica_groups=[list(range(NCORES))],
                        ins=[yt_loc[b][:].opt()], outs=[yt_ag[b][:].opt()])

            # ---- Phase 4: output projection (transposed), per batch ----
            with tc.tile_pool(name="fp", bufs=4) as fp, \
                 tc.tile_pool(name="fps", bufs=2, space="PSUM") as fps:
                for b in (range(B) if 4 in phases else ()):
                    for tbo in range(T // TB):
                        o_ps = [fps.tile([P, TB], f32, tag=f"o{i}",
                                         name=f"o{i}")
                                for i in range(2)]
                        for i in range(2):
                            for hf in range(2):
                                fs = slice(hf * TQ, (hf + 1) * TQ)
                                nc.tensor.matmul(
                                    o_ps[i][:, fs],
                                    bpt_sb[:, i * P:(i + 1) * P],
                                    ones_sb[:, fs], start=True, stop=False,
                                    skip_group_check=True)
                        for c in range(KC):
                            yt_t = fp.tile([P, TB], bf16, tag="yt")
                            eng = nc.sync if c % 2 == 0 else nc.scalar
                            eng.dma_start(
                                yt_t[:], yt_ag[b][c * P:(c + 1) * P,
                                                  tbo * TB:(tbo + 1) * TB])
                            for i in range(2):
                                for hf in range(2):
                                    fs = slice(hf * TQ, (hf + 1) * TQ)
                                    nc.tensor.matmul(
                                        o_ps[i][:, fs],
                                        wp_sb[:, c, i * P:(i + 1) * P],
                                        yt_t[:, fs], start=False,
                                        stop=(c == KC - 1),
                                        skip_group_check=True)
                        for i in range(2):
                            o_sb = fp.tile([P, TB], f32, tag="ob")
                            nc.vector.tensor_copy(o_sb[:], o_ps[i][:])
                            nc.sync.dma_start(
                                out_d[i * P:(i + 1) * P,
                                      b * T + tbo * TB: b * T + (tbo + 1) * TB],
                                o_sb[:])

    split_multi_waits(nc)
    return nc


_NC_CACHE = None


def _get_nc():
    global _NC_CACHE
    if _NC_CACHE is None:
        _NC_CACHE = build()
    return _NC_CACHE


def make_in_maps(x, wq, wk, wv, wp, bp):
    b16 = ml_dtypes.bfloat16
    x = np.asarray(x, dtype=np.float32)
    xt = np.ascontiguousarray(x.reshape(BT, C).T).astype(b16)
    kk = np.arange(P)[:, None]
    qq = np.arange(P)[None, :]
    tri = (kk <= qq).astype(np.float32)
    mask4 = np.tile(tri, (1, 4)).astype(b16)
    esel = np.zeros((32, 32 * HD), dtype=np.float32)
    for s in range(32):
        esel[s, s * HD:(s + 1) * HD] = 1.0
    esel = esel.astype(b16)
    ident = np.eye(P, dtype=np.float32).astype(b16)
    ones = np.ones((1, TB), dtype=np.float32).astype(b16)
    in_maps = []
    for i in range(NCORES):
        cs = slice(i * CQ, (i + 1) * CQ)
        ks = slice(i * HD, (i + 1) * HD)
        wkv = np.concatenate(
            [np.asarray(wk)[:, ks], np.asarray(wv)[:, ks]], axis=1)
        in_maps.append({
            "xt": xt,
            "wq": np.ascontiguousarray(np.asarray(wq, np.float32)[:, cs]).astype(b16),
            "wkv": np.ascontiguousarray(wkv.astype(np.float32)).astype(b16),
            "wp": np.ascontiguousarray(np.asarray(wp, np.float32)[:, cs]).astype(b16),
            "bpt": np.asarray(bp, np.float32)[None, cs].astype(b16),
            "ones": ones,
            "mask4": mask4,
            "esel": esel,
            "ident": ident,
        })
    return in_maps


def kernel(x, wq, wk, wv, wp, bp, _trace=False):
    from concourse.bass_utils import run_bass_kernel_spmd
    nc = _get_nc()
    in_maps = make_in_maps(x, wq, wk, wv, wp, bp)
    res = run_bass_kernel_spmd(nc, in_maps, list(range(NCORES)), trace=_trace)
    out = np.concatenate([res.results[i]["out"] for i in range(NCORES)], axis=0)
    out = np.ascontiguousarray(out.T).reshape(B, T, C).astype(np.float32)
    if _trace:
        return out, res
    return out


# revision 3
# speedup vs baseline: 1.1037x; 1.1037x over previous
"""GQA causal attention (B=2, T=2048, C=2048, 32 Q heads, 8 KV heads) on 8
Trainium2 NeuronCores — v2.

Sharding: tensor-parallel over KV-head groups (core i owns KV head i and its
4 query heads). All matmuls in bf16 (f32 PSUM accumulation): measured end-to-
end rel-err ~4e-3 vs the 2e-2 gate.

v2 structure (vs the f32r v1):
  - bf16 everywhere halves DMA and enables 1024-col moving operands + FWL.
  - Attention packs all 4 query heads into one scores tile [k, (pair,head,q)]
    so each kv-block stationary serves 1024-col matmuls; exp runs on 2048-col
    ACTIVATEs (the Act engine is the phase bottleneck at ~(N+352)/1.2 ns).
  - Causal diagonal blocks are trimmed at 128-col granularity (MMs, exp and
    the AV accumulation all skip the dead quarter-triangle).
  - Normalize uses a selector-matmul broadcast of 1/l (no per-row DMAs).
  - The AllGather is split per batch and overlaps the other batch's compute;
    the output projection runs transposed (out [oc, tok]) so wp chunks are
    stationary and yt tiles stream as 1024-col moving operands. Host
    transposes the final result.
"""

import sys

sys.path.insert(0, "/opt/trn_rl_repo")

import numpy as np
import ml_dtypes

import concourse.bass as bass
import concourse.mybir as mybir
import concourse.tile as tile

P = 128
B, T, C = 2, 2048, 2048
BT = B * T            # 4096
NH, NKV = 32, 8
HD = C // NH          # 64
G = NH // NKV         # 4 q heads per core
CQ = G * HD           # 256 q/out channels per core
KC = C // P           # 16 contraction chunks
TB = 1024             # phase-1/4 token block
TQ = 512              # attention q-chunk
NCORES = 8
NKB = T // P          # 16 kv blocks per batch

f32 = mybir.dt.float32
bf16 = mybir.dt.bfloat16
EXP = mybir.ActivationFunctionType.Exp
SCALE = float(HD) ** -0.5


def split_multi_waits(nc):
    """Walrus codegen allows only one sync-wait per engine instruction; move
    extras onto standalone same-engine EventSemaphore waits placed before."""
    for fn in nc.m.functions:
        for bb in fn.blocks:
            out = []
            for inst in bb.instructions:
                si = inst.sync_info
                if si is not None and si.on_wait and len(si.on_wait) > 1:
                    waits = list(si.on_wait)
                    for j, w in enumerate(waits[:-1]):
                        nop = mybir.InstEventSemaphore(
                            name=f"{inst.name}-ws{j}", ins=[], outs=[],
                            engine=inst.engine)
                        nop.sync_info = mybir.SyncInfo(on_wait=[w], on_update=[])
                        out.append(nop)
                    inst.sync_info = mybir.SyncInfo(
                        on_wait=[waits[-1]], on_update=list(si.on_update))
                out.append(inst)
            try:
                bb.instructions[:] = out
            except TypeError:
                bb.instructions.clear()
                bb.instructions.extend(out)


def build(phases=(1, 2, 3, 4)):
    nc = bass.Bass(num_devices=NCORES)

    xt_d = nc.dram_tensor("xt", [C, BT], bf16, kind="ExternalInput")
    wq_d = nc.dram_tensor("wq", [C, CQ], bf16, kind="ExternalInput")
    wkv_d = nc.dram_tensor("wkv", [C, P], bf16, kind="ExternalInput")
    wp_d = nc.dram_tensor("wp", [C, CQ], bf16, kind="ExternalInput")
    bpt_d = nc.dram_tensor("bpt", [1, CQ], bf16, kind="ExternalInput")
    ones_d = nc.dram_tensor("ones", [1, TB], bf16, kind="ExternalInput")
    mask_d = nc.dram_tensor("mask4", [P, 4 * P], bf16, kind="ExternalInput")
    esel_d = nc.dram_tensor("esel", [32, 32 * HD], bf16, kind="ExternalInput")
    idn_d = nc.dram_tensor("ident", [P, P], bf16, kind="ExternalInput")
    out_d = nc.dram_tensor("out", [CQ, BT], f32, kind="ExternalOutput")

    with tile.TileContext(nc) as tc, \
         nc.allow_low_precision(reason="bf16 pipeline validated vs 2e-2 gate"):
        with tc.tile_pool(name="res", bufs=1) as res, \
             tc.tile_pool(name="dram", bufs=1, space="DRAM") as dp:
            # constants / weights
            wq_sb = res.tile([P, KC, CQ], bf16)
            nc.sync.dma_start(wq_sb[:], wq_d.rearrange("(o p) n -> p o n", p=P))
            wkv_sb = res.tile([P, KC, P], bf16)
            nc.scalar.dma_start(wkv_sb[:], wkv_d.rearrange("(o p) n -> p o n", p=P))
            idn_sb = res.tile([P, P], bf16)
            nc.scalar.dma_start(idn_sb[:], idn_d[:, :])
            mask_sb = res.tile([P, 4 * P], bf16)
            nc.scalar.dma_start(mask_sb[:], mask_d[:, :])
            esel_sb = res.tile([32, 32 * HD], bf16)
            nc.scalar.dma_start(esel_sb[:], esel_d[:, :])
            ones_sb = res.tile([1, TB], bf16)
            nc.scalar.dma_start(ones_sb[:], ones_d[:, :])
            bpt_sb = res.tile([1, CQ], bf16)
            nc.scalar.dma_start(bpt_sb[:], bpt_d[:, :])
            wp_sb = res.tile([P, KC, CQ], bf16)

            # long-lived activations (both batches)
            # qP[pr]: [64, b, h, qc, TQ] for the head pair pr (heads 2pr, 2pr+1)
            qP = [res.tile([HD, B, 2, 4, TQ], bf16, name=f"qp{pr}")
                  for pr in range(2)]
            kT = res.tile([HD, B, NKB, P], bf16)
            va = res.tile([P, B, NKB, HD + 2], bf16)   # v natural + ones col
            yU = res.tile([HD + 1, B, 4, 4 * TQ], bf16)  # unnormalized y + l row
            lA = res.tile([32, TQ], bf16)              # l rows, slot-major
            rA = res.tile([32, TQ], bf16)
            yt_loc = [dp.tile([CQ, T], bf16, name=f"ytl{b}") for b in range(B)]
            yt_ag = [dp.tile([NCORES * CQ, T], bf16, addr_space="Shared",
                             name=f"ytag{b}") for b in range(B)]

            yn_all = res.tile([HD, 4, 4, TQ], bf16)
            nc.vector.memset(va[:, :, :, HD:HD + 1], 1.0)
            nc.vector.memset(lA[:, :], 1.0)

            # PE warmup: dummy matmuls on uninitialized SBUF ramp the HAM
            # clock gate to 8/8 while the first weight/x DMAs are in flight.
            warm_sb = res.tile([P, TQ], bf16)
            nc.vector.memset(warm_sb[:, :], 1.0)
            with tc.tile_pool(name="wps", bufs=1, space="PSUM") as wps:
                w_ps = wps.tile([P, TQ], f32)
                for _ in range(14):
                    nc.tensor.matmul(w_ps[:], warm_sb[:, 0:P], warm_sb[:],
                                     start=True, stop=True,
                                     skip_group_check=True)
            nc.vector.memset(va[:, :, :, HD + 1:HD + 2], 0.0)

            for b in range(B):
                if 1 not in phases:
                    break
                # ---- Phase 1: q/k/v projections for batch b ----
                with tc.tile_pool(name=f"xp{b}", bufs=5) as xp, \
                     tc.tile_pool(name=f"pps{b}", bufs=1, space="PSUM") as pps, \
                     tc.tile_pool(name=f"tps{b}", bufs=2, space="PSUM") as tps:
                    for tb in range(T // TB):
                        q01_ps = pps.tile([P, TB], f32, tag="q01")
                        q23_ps = pps.tile([P, TB], f32, tag="q23")
                        kv_ps = pps.tile([P, TB], f32, tag="kv")
                        for c4 in range(KC // 4):
                            xt_t = xp.tile([P, 4, TB], bf16, tag="xt")
                            eng = nc.sync if c4 % 2 == 0 else nc.scalar
                            eng.dma_start(
                                xt_t[:],
                                xt_d[c4 * 4 * P:(c4 + 1) * 4 * P,
                                     b * T + tb * TB: b * T + (tb + 1) * TB]
                                .rearrange("(o p) n -> p o n", p=P))
                            for ci in range(4):
                                c = c4 * 4 + ci
                                for hf in range(2):
                                    fs = slice(hf * TQ, (hf + 1) * TQ)
                                    nc.tensor.matmul(q01_ps[:, fs],
                                                     wq_sb[:, c, 0:P],
                                                     xt_t[:, ci, fs],
                                                     start=(c == 0),
                                                     stop=(c == KC - 1))
                                    nc.tensor.matmul(q23_ps[:, fs],
                                                     wq_sb[:, c, P:CQ],
                                                     xt_t[:, ci, fs],
                                                     start=(c == 0),
                                                     stop=(c == KC - 1))
                                    nc.tensor.matmul(kv_ps[:, fs],
                                                     wkv_sb[:, c, :],
                                                     xt_t[:, ci, fs],
                                                     start=(c == 0),
                                                     stop=(c == KC - 1))
                        qsl = (slice(None), b, slice(None),
                               slice(2 * tb, 2 * tb + 2), slice(None))
                        nc.vector.tensor_copy(
                            qP[0][:, b, 0, 2 * tb:2 * tb + 2, :].rearrange(
                                "p a n -> p (a n)"), q01_ps[0:HD, :])
                        nc.vector.tensor_copy(
                            qP[0][:, b, 1, 2 * tb:2 * tb + 2, :].rearrange(
                                "p a n -> p (a n)"), q01_ps[HD:P, :])
                        nc.vector.tensor_copy(
                            qP[1][:, b, 0, 2 * tb:2 * tb + 2, :].rearrange(
                                "p a n -> p (a n)"), q23_ps[0:HD, :])
                        nc.vector.tensor_copy(
                            qP[1][:, b, 1, 2 * tb:2 * tb + 2, :].rearrange(
                                "p a n -> p (a n)"), q23_ps[HD:P, :])
                        nc.vector.tensor_copy(
                            kT[:, b, 8 * tb:8 * (tb + 1), :].rearrange(
                                "p k n -> p (k n)"),
                            kv_ps[0:HD, :])
                        vs_t = xp.tile([HD, TB], bf16, tag="vs")
                        nc.vector.tensor_copy(vs_t[:], kv_ps[HD:P, :])
                        for k8 in range(TB // P):
                            kb = tb * (TB // P) + k8
                            vt_ps = tps.tile([P, HD], bf16, tag="vt")
                            nc.tensor.transpose(
                                vt_ps[:], vs_t[:, k8 * P:(k8 + 1) * P],
                                idn_sb[0:HD, 0:HD])
                            nc.vector.tensor_copy(va[:, b, kb, 0:HD], vt_ps[:])

                if 2 not in phases:
                    continue
                # ---- Phase 2: attention for batch b, all 4 heads packed ----
                with tc.tile_pool(name=f"aps{b}", bufs=2, space="PSUM") as aps, \
                     tc.tile_pool(name=f"yps{b}", bufs=1, space="PSUM") as yps, \
                     tc.tile_pool(name=f"ep{b}", bufs=4) as ep:
                    for qc in range(4):
                        y_ps = yps.tile([HD + 2, 4, TQ], f32, tag="y")
                        nkb = 4 * qc + 4
                        for kb in range(nkb):
                            j = kb - 4 * qc
                            cols = slice(0, TQ) if j < 1 else slice(j * P, TQ)
                            sps = []
                            exs = []
                            for pr in range(2):
                                s_ps = aps.tile([P, 2, TQ], f32, tag="s")
                                for h in range(2):
                                    nc.tensor.matmul(
                                        s_ps[:, h, cols],
                                        kT[:, b, kb, :],
                                        qP[pr][:, b, h, qc, cols],
                                        start=True, stop=True,
                                        skip_group_check=True)
                                sps.append(s_ps)
                            for pr in range(2):
                                ex = ep.tile([P, 2, TQ], bf16, tag="ex")
                                nc.scalar.activation(ex[:, :, cols],
                                                     sps[pr][:, :, cols], EXP,
                                                     scale=SCALE)
                                if j >= 0:
                                    nc.vector.tensor_mul(
                                        ex[:, :, j * P:(j + 1) * P],
                                        ex[:, :, j * P:(j + 1) * P],
                                        mask_sb[:, 0:2 * P].rearrange(
                                            "p (g n) -> p g n", g=2))
                                exs.append(ex)
                            for pr in range(2):
                                for h in range(2):
                                    nc.tensor.matmul(
                                        y_ps[:, 2 * pr + h, cols],
                                        va[:, b, kb, :],
                                        exs[pr][:, h, cols],
                                        start=(kb == 0),
                                        stop=(kb == nkb - 1),
                                        skip_group_check=True)
                        nc.vector.tensor_copy(
                            yU[:, b, qc, :].rearrange("p (g n) -> p g n", g=4),
                            y_ps[0:HD + 1, :, :])
                        for hh in range(4):
                            slot = (b * 4 + qc) * 4 + hh
                            nc.sync.dma_start(
                                lA[slot:slot + 1, :],
                                yU[HD:HD + 1, b, qc,
                                   hh * TQ:(hh + 1) * TQ])

                if 3 not in phases:
                    continue
                # ---- Phase 3: normalize + per-batch AllGather ----
                with tc.tile_pool(name=f"np{b}", bufs=4) as npo, \
                     tc.tile_pool(name=f"bps{b}", bufs=2, space="PSUM") as bps:
                    if b == 0:
                        nc.scalar.dma_start(
                            wp_sb[:], wp_d.rearrange("(o p) n -> p o n", p=P))
                    nc.vector.reciprocal(rA[:, :], lA[:, :])
                    for qc in range(4):
                        for hh in range(4):
                            slot = b * 16 + qc * 4 + hh
                            qh = hh  # head index within core (pr*2+h)
                            bc_ps = bps.tile([HD, TQ], f32, tag="bc")
                            nc.tensor.matmul(
                                bc_ps[:], esel_sb[:, slot * HD:(slot + 1) * HD],
                                rA[:, :], start=True, stop=True)
                            nc.vector.tensor_mul(
                                yn_all[:, qh, qc, :],
                                yU[0:HD, b, qc, hh * TQ:(hh + 1) * TQ],
                                bc_ps[:])
                    nc.scalar.dma_start(
                        yt_loc[b][:, :].rearrange(
                            "(qh d) (qc c) -> d qh qc c", d=HD, c=TQ),
                        yn_all[:, :, :, :])
                    nc.gpsimd.collective_compute(
                        "AllGather", mybir.AluOpType.bypass,
                        replica_groups=[list(range(NCORES))],
                        ins=[yt_loc[b][:].opt()], outs=[yt_ag[b][:].opt()])

            # ---- Phase 4: output projection (transposed), per batch ----
            with tc.tile_pool(name="fp", bufs=4) as fp, \
                 tc.tile_pool(name="fps", bufs=2, space="PSUM") as fps:
                for b in (range(B) if 4 in phases else ()):
                    for tbo in range(T // TB):
                        o_ps = [fps.tile([P, TB], f32, tag=f"o{i}",
                                         name=f"o{i}")
                                for i in range(2)]
                        for i in range(2):
                            for hf in range(2):
                                fs = slice(hf * TQ, (hf + 1) * TQ)
                                nc.tensor.matmul(
                                    o_ps[i][:, fs],
                                    bpt_sb[:, i * P:(i + 1) * P],
                                    ones_sb[:, fs], start=True, stop=False,
                                    skip_group_check=True)
                        for c2 in range(KC // 2):
                            yt_t = fp.tile([P, 2, TB], bf16, tag="yt")
                            eng = nc.sync if c2 % 2 == 0 else nc.scalar
                            eng.dma_start(
                                yt_t[:],
                                yt_ag[b][c2 * 2 * P:(c2 + 1) * 2 * P,
                                         tbo * TB:(tbo + 1) * TB].rearrange(
                                    "(o p) n -> p o n", p=P))
                            for ci in range(2):
                                c = c2 * 2 + ci
                                for i in range(2):
                                    for hf in range(2):
                                        fs = slice(hf * TQ, (hf + 1) * TQ)
                                        nc.tensor.matmul(
                                            o_ps[i][:, fs],
                                            wp_sb[:, c, i * P:(i + 1) * P],
                                            yt_t[:, ci, fs], start=False,
                                            stop=(c == KC - 1),
                                            skip_group_check=True)
                        for i in range(2):
                            o_sb = fp.tile([P, TB], f32, tag="ob")
                            nc.vector.tensor_copy(o_sb[:], o_ps[i][:])
                            nc.sync.dma_start(
                                out_d[i * P:(i + 1) * P,
                                      b * T + tbo * TB: b * T + (tbo + 1) * TB],
                                o_sb[:])

    split_multi_waits(nc)
    return nc


_NC_CACHE = None


def _get_nc():
    global _NC_CACHE
    if _NC_CACHE is None:
        _NC_CACHE = build()
    return _NC_CACHE


def make_in_maps(x, wq, wk, wv, wp, bp):
    b16 = ml_dtypes.bfloat16
    x = np.asarray(x, dtype=np.float32)
    xt = np.ascontiguousarray(x.reshape(BT, C).T).astype(b16)
    kk = np.arange(P)[:, None]
    qq = np.arange(P)[None, :]
    tri = (kk <= qq).astype(np.float32)
    mask4 = np.tile(tri, (1, 4)).astype(b16)
    esel = np.zeros((32, 32 * HD), dtype=np.float32)
    for s in range(32):
        esel[s, s * HD:(s + 1) * HD] = 1.0
    esel = esel.astype(b16)
    ident = np.eye(P, dtype=np.float32).astype(b16)
    ones = np.ones((1, TB), dtype=np.float32).astype(b16)
    in_maps = []
    for i in range(NCORES):
        cs = slice(i * CQ, (i + 1) * CQ)
        ks = slice(i * HD, (i + 1) * HD)
        wkv = np.concatenate(
            [np.asarray(wk)[:, ks], np.asarray(wv)[:, ks]], axis=1)
        in_maps.append({
            "xt": xt,
            "wq": np.ascontiguousarray(np.asarray(wq, np.float32)[:, cs]).astype(b16),
            "wkv": np.ascontiguousarray(wkv.astype(np.float32)).astype(b16),
            "wp": np.ascontiguousarray(np.asarray(wp, np.float32)[:, cs]).astype(b16),
            "bpt": np.asarray(bp, np.float32)[None, cs].astype(b16),
            "ones": ones,
            "mask4": mask4,
            "esel": esel,
            "ident": ident,
        })
    return in_maps


def kernel(x, wq, wk, wv, wp, bp, _trace=False):
    from concourse.bass_utils import run_bass_kernel_spmd
    nc = _get_nc()
    in_maps = make_in_maps(x, wq, wk, wv, wp, bp)
    res = run_bass_kernel_spmd(nc, in_maps, list(range(NCORES)), trace=_trace)
    out = np.concatenate([res.results[i]["out"] for i in range(NCORES)], axis=0)
    out = np.ascontiguousarray(out.T).reshape(B, T, C).astype(np.float32)
    if _trace:
        return out, res
    return out
